# revision 21
# baseline (speedup 1.0000x reference)
"""Trainium2 Bass kernel for nn_EntityMapping (P=16 independent MLPs over a
shared entity batch).

Sharding: the 16 partition-MLPs are split across 8 NeuronCores (2 per core,
expert-parallel); the embedding batch is replicated. Activations are kept
feature-major [feature, batch] on-chip so every layer is a chain of
128x128-stationary matmuls with the batch streaming through the PE array.
Matmuls run in float32r (full-rate fp32 on TRN2's PE at N>=256; inputs are
rounded to fp32r by DVE producer ops as walrus requires).

Weights are host-packed in PE consumption order (p,j,k) and streamed in
eighths, DMA-issued and DVE-rounded in exactly the order the PE consumes
them, so the first matmul fires ~12us in and never starves; ~16 junk
matmuls on a memset tile warm the PE clock (HAM) during the load window.
"""

import numpy as np

try:
    import concourse.bass as bass  # noqa: F401
except ImportError:  # harness runs kernel.py from a bare directory
    import sys

    sys.path.insert(0, "/opt/trn_rl_repo")

import concourse.mybir as mybir
import concourse.tile as tile
from concourse import bacc
from concourse.bass_utils import run_bass_kernel_spmd

F32 = mybir.dt.float32
F32R = mybir.dt.float32r
RELU = mybir.ActivationFunctionType.Relu
SIGMOID = mybir.ActivationFunctionType.Sigmoid
COPY = mybir.ActivationFunctionType.Copy

P_TOTAL = 16  # independent MLP partitions
E = 512  # entity/embedding dim
H = 512  # hidden dim
N = 8192  # batch (entities)
N_CORES = 8
P_PER = P_TOTAL // N_CORES  # 2 MLPs per core
KC = E // 128  # 4 contraction chunks per layer
JC = H // 128  # 4 output-feature chunks per layer
NCH = 512  # batch columns per n-chunk (= fp32 moving-operand max = PSUM bank)
NCHUNKS = N // NCH  # 16
NW = P_PER * KC * JC  # 32 weight tiles per layer
PIECE = JC * 128  # weight piece = one (p,j) group of KC tiles = 512 cols
WARMUP_MM = 16  # junk matmuls to warm the PE clock during weight load


def _build():
    nc = bacc.Bacc(
        "TRN2", target_bir_lowering=False, debug=False, num_devices=N_CORES
    )
    # All inputs pre-packed on host into SBUF-layout [128, cols]:
    eT_dram = nc.dram_tensor("eT", [E, N], F32, kind="ExternalInput")
    w0_dram = nc.dram_tensor("w0", [128, NW * 128], F32, kind="ExternalInput")
    w1_dram = nc.dram_tensor("w1", [128, NW * 128], F32, kind="ExternalInput")
    b0_dram = nc.dram_tensor("b0", [128, P_PER * JC], F32, kind="ExternalInput")
    b1_dram = nc.dram_tensor("b1", [128, P_PER * JC], F32, kind="ExternalInput")
    w2_dram = nc.dram_tensor("w2", [128, P_PER * KC], F32, kind="ExternalInput")
    b2_dram = nc.dram_tensor("b2", [1, P_PER], F32, kind="ExternalInput")
    out_dram = nc.dram_tensor("out", [P_PER, N], F32, kind="ExternalOutput")

    # eT viewed as [ki, k, n] for per-k chunk DMAs
    eT_v = eT_dram.rearrange("(k ki) n -> ki k n", ki=128)

    with tile.TileContext(nc) as tc:
        with (
            tc.tile_pool(name="wconst", bufs=1) as wconst,
            tc.tile_pool(name="wstage", bufs=1) as wstage,
            tc.tile_pool(name="warm", bufs=1) as warm_pool,
            tc.tile_pool(name="et", bufs=3) as et_pool,
            tc.tile_pool(name="etr", bufs=3) as etr_pool,
            tc.tile_pool(name="act", bufs=2) as act_pool,
            tc.tile_pool(name="osb", bufs=4) as out_pool,
            tc.tile_pool(name="mmps", bufs=6, space="PSUM") as ps_mm,
            tc.tile_pool(name="l2ps", bufs=2, space="PSUM") as ps_l2,
        ):
            # persistent rounded weights + staging
            w0_r = wconst.tile([128, NW, 128], F32R, tag="w0r")
            w1_r = wconst.tile([128, NW, 128], F32R, tag="w1r")
            w0_rf = w0_r[:].rearrange("p a b -> p (a b)")
            w1_rf = w1_r[:].rearrange("p a b -> p (a b)")
            w0_st = wstage.tile([128, NW * 128], F32, tag="s0")
            w1_st = wstage.tile([128, NW * 128], F32, tag="s1")

            def w_dma(st, wd, q):  # stream piece q of a weight layer
                nc.sync.dma_start(
                    st[:, q * PIECE : (q + 1) * PIECE],
                    wd[:, q * PIECE : (q + 1) * PIECE],
                )

            def w_round(st, wrf, q, alt=False):  # f32r-round piece q
                if alt:
                    nc.scalar.activation(
                        wrf[:, q * PIECE : (q + 1) * PIECE],
                        st[:, q * PIECE : (q + 1) * PIECE],
                        COPY,
                    )
                else:
                    nc.vector.tensor_copy(
                        wrf[:, q * PIECE : (q + 1) * PIECE],
                        st[:, q * PIECE : (q + 1) * PIECE],
                    )

            # --- PE warmup: junk matmuls on a memset tile so HAM is at
            # K=8/8 when the first real matmul issues ---
            wm_f = warm_pool.tile([128, 640], F32, tag="wmf")
            nc.gpsimd.memset(wm_f[:], 0.0)
            wm_r = warm_pool.tile([128, 640], F32R, tag="wmr")
            nc.vector.tensor_copy(wm_r[:], wm_f[:])
            ps_warm = ps_l2.tile([128, 512], F32, tag="l2")
            for i in range(WARMUP_MM):
                nc.tensor.matmul(
                    ps_warm[:],
                    wm_r[:, 0:128],
                    wm_r[:, 128:640],
                    start=(i == 0),
                    stop=(i == WARMUP_MM - 1),
                )

            # first weight piece + small constants up front
            w_dma(w0_st, w0_dram, 0)
            b0_sb = wconst.tile([128, P_PER * JC], F32, tag="b0")
            nc.sync.dma_start(b0_sb[:], b0_dram[:])
            b1_sb = wconst.tile([128, P_PER * JC], F32, tag="b1")
            nc.sync.dma_start(b1_sb[:], b1_dram[:])
            b2_sb = wconst.tile([1, P_PER], F32, tag="b2")
            nc.sync.dma_start(b2_sb[:], b2_dram[:])
            w2_sb = wconst.tile([128, P_PER * KC], F32, tag="w2sb")
            nc.sync.dma_start(w2_sb[:], w2_dram[:])
            w_round(w0_st, w0_rf, 0)
            # ones column for the L2 partition-reduction matmul
            ones_f = warm_pool.tile([128, 1], F32, tag="onef")
            nc.gpsimd.memset(ones_f[:], 1.0)
            ones_r = warm_pool.tile([128, 1], F32R, tag="oner")
            nc.vector.tensor_copy(ones_r[:], ones_f[:])
            # f32r w2 for the final chunk's direct-matmul L2 (shorter tail)
            w2_r = wconst.tile([128, P_PER * KC], F32R, tag="w2r")
            nc.vector.tensor_copy(w2_r[:], w2_sb[:])

            def l0(p, et):
                h = act_pool.tile([128, JC, NCH], F32R, tag="h")
                for j in range(JC):
                    ps = ps_mm.tile([128, NCH], F32, tag="mm")
                    for k in range(KC):
                        wi = (p * JC + j) * KC + k
                        nc.tensor.matmul(
                            ps[:], w0_r[:, wi, :], et[:, k, :],
                            start=(k == 0), stop=(k == KC - 1),
                        )
                    nc.scalar.activation(
                        h[:, j, :], ps[:], RELU,
                        bias=b0_sb[:, p * JC + j : p * JC + j + 1],
                    )
                return h

            def l12(p, h, n0, direct=False):
                # L1 + L2 fused: after each relu j, scale by w2[j] on DVE and
                # accumulate the k-sum g incrementally, so the per-chunk tail
                # is just mul+add+ones-matmul+sigmoid.
                # u[n] = sum_feat w2[feat]*h2[feat,n] = ones^T g.
                # direct=True (final chunk) reduces via 4 w2-matmuls instead,
                # skipping the DVE chain for a shorter kernel tail.
                h2 = act_pool.tile([128, JC, NCH], F32R, tag="h2")
                g = act_pool.tile([128, NCH], F32R, tag="g")
                r = ps_l2.tile([1, NCH], F32, tag="l2")
                for j in range(JC):
                    ps = ps_mm.tile([128, NCH], F32, tag="mm")
                    for k in range(KC):
                        wi = (p * JC + j) * KC + k
                        nc.tensor.matmul(
                            ps[:], w1_r[:, wi, :], h[:, k, :],
                            start=(k == 0), stop=(k == KC - 1),
                        )
                    nc.scalar.activation(
                        h2[:, j, :], ps[:], RELU,
                        bias=b1_sb[:, p * JC + j : p * JC + j + 1],
                    )
                    if direct:
                        nc.tensor.matmul(
                            r[:], w2_r[:, p * KC + j : p * KC + j + 1],
                            h2[:, j, :], start=(j == 0), stop=(j == JC - 1),
                        )
                        continue
                    nc.vector.tensor_scalar_mul(
                        h2[:, j, :], h2[:, j, :],
                        w2_sb[:, p * KC + j : p * KC + j + 1],
                    )
                    if j == 1:
                        nc.vector.tensor_add(g[:], h2[:, 0, :], h2[:, 1, :])
                    elif j > 1:
                        nc.vector.tensor_add(g[:], g[:], h2[:, j, :])
                if not direct:
                    nc.tensor.matmul(r[:], ones_r[:], g[:], start=True, stop=True)
                o = out_pool.tile([1, NCH], F32, tag="o")
                nc.scalar.activation(o[:], r[:], SIGMOID, bias=b2_sb[0:1, p : p + 1])
                nc.sync.dma_start(out_dram[p : p + 1, n0 : n0 + NCH], o[:])

            def load_et(c, eng=None):
                # et0 rides gpsimd (issues immediately); later chunks ride
                # sync BEHIND the weight pieces so they can't steal wire
                # bandwidth from w1 during the ramp.  During the ramp (c<2)
                # the f32r casts alternate DVE/ACT so they finish sooner.
                n0 = c * NCH
                et_f = et_pool.tile([128, KC, NCH], F32, tag="et")
                eng = eng or nc.sync
                for k in range(KC):
                    eng.dma_start(et_f[:, k, :], eT_v[:, k, n0 : n0 + NCH])
                et = etr_pool.tile([128, KC, NCH], F32R, tag="etr")
                for k in range(KC):
                    if c < 2 and k % 2 == 1:
                        nc.scalar.activation(et[:, k, :], et_f[:, k, :], COPY)
                    else:
                        nc.vector.tensor_copy(et[:, k, :], et_f[:, k, :])
                return et

            # --- chunk 0: L0 for both partitions first (needs only w0+et0),
            # giving the wire time to deliver w1; loads interleaved in
            # consumption order ---
            et0 = load_et(0, eng=nc.gpsimd)
            for q in range(1, P_PER * JC):  # w0 pieces 1..7
                w_dma(w0_st, w0_dram, q)
            for q in range(P_PER * JC):  # all w1 pieces
                w_dma(w1_st, w1_dram, q)
            for q in range(1, P_PER * JC):
                w_round(w0_st, w0_rf, q, alt=(q % 2 == 1))
            h_0 = l0(0, et0)
            h_1 = l0(1, et0)
            et1 = load_et(1)
            for q in range(P_PER * JC):
                w_round(w1_st, w1_rf, q, alt=(q % 2 == 1))
            l12(0, h_0, 0)
            l12(1, h_1, 0)

            # --- steady-state loop ---
            ets = {1: et1}
            for c in range(1, NCHUNKS):
                et = ets.pop(c)
                if c + 1 < NCHUNKS:
                    ets[c + 1] = load_et(c + 1)
                n0 = c * NCH
                for p in range(P_PER):
                    h = l0(p, et)
                    last = c == NCHUNKS - 1 and p == P_PER - 1
                    l12(p, h, n0, direct=last)

    nc.compile()
    return nc


_NC_CACHE = None


def _get_nc():
    global _NC_CACHE
    if _NC_CACHE is None:
        _NC_CACHE = _build()
    return _NC_CACHE


def _make_in_maps(e_embedding, W0, b0, W1, b1, W2, b2):
    e = np.asarray(e_embedding, dtype=np.float32)
    W0 = np.asarray(W0, dtype=np.float32)
    b0 = np.asarray(b0, dtype=np.float32)
    W1 = np.asarray(W1, dtype=np.float32)
    b1 = np.asarray(b1, dtype=np.float32)
    W2 = np.asarray(W2, dtype=np.float32)
    b2 = np.asarray(b2, dtype=np.float32)

    eT = np.ascontiguousarray(e.T)  # [E, N]
    in_maps = []
    for cid in range(N_CORES):
        sl = slice(P_PER * cid, P_PER * (cid + 1))
        # SBUF layout [ki, ((p j k) ji)] — PE consumption order
        w0t = np.ascontiguousarray(
            W0[sl]
            .reshape(P_PER, KC, 128, JC, 128)
            .transpose(2, 0, 3, 1, 4)
            .reshape(128, NW * 128)
        )
        w1t = np.ascontiguousarray(
            W1[sl]
            .reshape(P_PER, KC, 128, JC, 128)
            .transpose(2, 0, 3, 1, 4)
            .reshape(128, NW * 128)
        )
        # [ki, (p j)] bias columns
        b0t = np.ascontiguousarray(
            b0[sl].reshape(P_PER, JC, 128).transpose(2, 0, 1).reshape(128, -1)
        )
        b1t = np.ascontiguousarray(
            b1[sl].reshape(P_PER, JC, 128).transpose(2, 0, 1).reshape(128, -1)
        )
        w2t = np.ascontiguousarray(
            W2[sl, :, 0].reshape(P_PER, KC, 128).transpose(2, 0, 1).reshape(128, -1)
        )
        b2t = np.ascontiguousarray(b2[sl].reshape(1, P_PER))
        in_maps.append(
            {"eT": eT, "w0": w0t, "b0": b0t, "w1": w1t, "b1": b1t,
             "w2": w2t, "b2": b2t}
        )
    return in_maps


def kernel_with_results(trace=False, **inputs):
    nc = _get_nc()
    in_maps = _make_in_maps(**inputs)
    res = run_bass_kernel_spmd(
        nc, in_maps, core_ids=list(range(N_CORES)), trace=trace
    )
    full = np.concatenate([r["out"] for r in res.results], axis=0)  # [16, N]
    out = np.ascontiguousarray(full.T).astype(np.float32)  # [N, 16]
    return out, res


def kernel(**inputs):
    out, _ = kernel_with_results(trace=False, **inputs)
    return out


# revision 23
# speedup vs baseline: 1.0041x; 1.0041x over previous
"""Trainium2 Bass kernel for nn_EntityMapping (P=16 independent MLPs over a
shared entity batch).

Sharding: the 16 partition-MLPs are split across 8 NeuronCores (2 per core,
expert-parallel); the embedding batch is replicated. Activations are kept
feature-major [feature, batch] on-chip so every layer is a chain of
128x128-stationary matmuls with the batch streaming through the PE array.
Matmuls run in float32r (full-rate fp32 on TRN2's PE at N>=256; inputs are
rounded to fp32r by DVE producer ops as walrus requires).

Weights are host-packed in PE consumption order (p,j,k) and streamed in
eighths, DMA-issued and DVE-rounded in exactly the order the PE consumes
them, so the first matmul fires ~12us in and never starves; ~16 junk
matmuls on a memset tile warm the PE clock (HAM) during the load window.
"""

import os
import sys

import numpy as np

if "jax" not in sys.modules and os.environ.get("JAX_PLATFORMS") == "cpu":
    # don't let a cpu pin hide the axon/neuron backend the kernel runs on
    os.environ["JAX_PLATFORMS"] = ""

try:
    import concourse.bass as bass  # noqa: F401
except ImportError:  # harness runs kernel.py from a bare directory
    sys.path.insert(0, "/opt/trn_rl_repo")

import concourse.mybir as mybir
import concourse.tile as tile
from concourse import bacc
from concourse.bass_utils import run_bass_kernel_spmd

F32 = mybir.dt.float32
F32R = mybir.dt.float32r
RELU = mybir.ActivationFunctionType.Relu
SIGMOID = mybir.ActivationFunctionType.Sigmoid
COPY = mybir.ActivationFunctionType.Copy

P_TOTAL = 16  # independent MLP partitions
E = 512  # entity/embedding dim
H = 512  # hidden dim
N = 8192  # batch (entities)
N_CORES = 8
P_PER = P_TOTAL // N_CORES  # 2 MLPs per core
KC = E // 128  # 4 contraction chunks per layer
JC = H // 128  # 4 output-feature chunks per layer
NCH = 512  # batch columns per n-chunk (= fp32 moving-operand max = PSUM bank)
NCHUNKS = N // NCH  # 16
NW = P_PER * KC * JC  # 32 weight tiles per layer
PIECE = JC * 128  # weight piece = one (p,j) group of KC tiles = 512 cols
WARMUP_MM = 16  # junk matmuls to warm the PE clock during weight load


def _build():
    nc = bacc.Bacc(
        "TRN2", target_bir_lowering=False, debug=False, num_devices=N_CORES
    )
    # All inputs pre-packed on host into SBUF-layout [128, cols]:
    eT_dram = nc.dram_tensor("eT", [E, N], F32, kind="ExternalInput")
    w0_dram = nc.dram_tensor("w0", [128, NW * 128], F32, kind="ExternalInput")
    w1_dram = nc.dram_tensor("w1", [128, NW * 128], F32, kind="ExternalInput")
    b0_dram = nc.dram_tensor("b0", [128, P_PER * JC], F32, kind="ExternalInput")
    b1_dram = nc.dram_tensor("b1", [128, P_PER * JC], F32, kind="ExternalInput")
    w2_dram = nc.dram_tensor("w2", [128, P_PER * KC], F32, kind="ExternalInput")
    b2_dram = nc.dram_tensor("b2", [1, P_PER], F32, kind="ExternalInput")
    out_dram = nc.dram_tensor("out", [P_PER, N], F32, kind="ExternalOutput")

    # eT viewed as [ki, k, n] for per-k chunk DMAs
    eT_v = eT_dram.rearrange("(k ki) n -> ki k n", ki=128)

    with tile.TileContext(nc) as tc:
        with (
            tc.tile_pool(name="wconst", bufs=1) as wconst,
            tc.tile_pool(name="wstage", bufs=1) as wstage,
            tc.tile_pool(name="warm", bufs=1) as warm_pool,
            tc.tile_pool(name="et", bufs=3) as et_pool,
            tc.tile_pool(name="etr", bufs=3) as etr_pool,
            tc.tile_pool(name="act", bufs=2) as act_pool,
            tc.tile_pool(name="osb", bufs=4) as out_pool,
            tc.tile_pool(name="mmps", bufs=6, space="PSUM") as ps_mm,
            tc.tile_pool(name="l2ps", bufs=2, space="PSUM") as ps_l2,
        ):
            # persistent rounded weights + staging
            w0_r = wconst.tile([128, NW, 128], F32R, tag="w0r")
            w1_r = wconst.tile([128, NW, 128], F32R, tag="w1r")
            w0_rf = w0_r[:].rearrange("p a b -> p (a b)")
            w1_rf = w1_r[:].rearrange("p a b -> p (a b)")
            w0_st = wstage.tile([128, NW * 128], F32, tag="s0")
            w1_st = wstage.tile([128, NW * 128], F32, tag="s1")

            def w_dma(st, wd, q):  # stream piece q of a weight layer
                nc.sync.dma_start(
                    st[:, q * PIECE : (q + 1) * PIECE],
                    wd[:, q * PIECE : (q + 1) * PIECE],
                )

            def w_round(st, wrf, q, alt=False):  # f32r-round piece q
                if alt:
                    nc.scalar.activation(
                        wrf[:, q * PIECE : (q + 1) * PIECE],
                        st[:, q * PIECE : (q + 1) * PIECE],
                        COPY,
                    )
                else:
                    nc.vector.tensor_copy(
                        wrf[:, q * PIECE : (q + 1) * PIECE],
                        st[:, q * PIECE : (q + 1) * PIECE],
                    )

            # --- PE warmup: junk matmuls on a memset tile so HAM is at
            # K=8/8 when the first real matmul issues ---
            wm_f = warm_pool.tile([128, 640], F32, tag="wmf")
            nc.gpsimd.memset(wm_f[:], 0.0)
            wm_r = warm_pool.tile([128, 640], F32R, tag="wmr")
            nc.vector.tensor_copy(wm_r[:], wm_f[:])
            ps_warm = ps_l2.tile([128, 512], F32, tag="l2")
            for i in range(WARMUP_MM):
                nc.tensor.matmul(
                    ps_warm[:],
                    wm_r[:, 0:128],
                    wm_r[:, 128:640],
                    start=(i == 0),
                    stop=(i == WARMUP_MM - 1),
                )

            # first weight piece + small constants up front
            w_dma(w0_st, w0_dram, 0)
            b0_sb = wconst.tile([128, P_PER * JC], F32, tag="b0")
            nc.sync.dma_start(b0_sb[:], b0_dram[:])
            b1_sb = wconst.tile([128, P_PER * JC], F32, tag="b1")
            nc.sync.dma_start(b1_sb[:], b1_dram[:])
            b2_sb = wconst.tile([1, P_PER], F32, tag="b2")
            nc.sync.dma_start(b2_sb[:], b2_dram[:])
            w2_sb = wconst.tile([128, P_PER * KC], F32, tag="w2sb")
            nc.sync.dma_start(w2_sb[:], w2_dram[:])
            w_round(w0_st, w0_rf, 0)
            # ones column for the L2 partition-reduction matmul
            ones_f = warm_pool.tile([128, 1], F32, tag="onef")
            nc.gpsimd.memset(ones_f[:], 1.0)
            ones_r = warm_pool.tile([128, 1], F32R, tag="oner")
            nc.vector.tensor_copy(ones_r[:], ones_f[:])
            # f32r w2 for the final chunk's direct-matmul L2 (shorter tail)
            w2_r = wconst.tile([128, P_PER * KC], F32R, tag="w2r")
            nc.vector.tensor_copy(w2_r[:], w2_sb[:])

            def l0(p, et):
                h = act_pool.tile([128, JC, NCH], F32R, tag="h")
                for j in range(JC):
                    ps = ps_mm.tile([128, NCH], F32, tag="mm")
                    for k in range(KC):
                        wi = (p * JC + j) * KC + k
                        nc.tensor.matmul(
                            ps[:], w0_r[:, wi, :], et[:, k, :],
                            start=(k == 0), stop=(k == KC - 1),
                        )
                    nc.scalar.activation(
                        h[:, j, :], ps[:], RELU,
                        bias=b0_sb[:, p * JC + j : p * JC + j + 1],
                    )
                return h

            def l12(p, h, n0, direct=False):
                # L1 + L2 fused: after each relu j, scale by w2[j] on DVE and
                # accumulate the k-sum g incrementally, so the per-chunk tail
                # is just mul+add+ones-matmul+sigmoid.
                # u[n] = sum_feat w2[feat]*h2[feat,n] = ones^T g.
                # direct=True (final chunk) reduces via 4 w2-matmuls instead,
                # skipping the DVE chain for a shorter kernel tail.
                h2 = act_pool.tile([128, JC, NCH], F32R, tag="h2")
                g = act_pool.tile([128, NCH], F32R, tag="g")
                r = ps_l2.tile([1, NCH], F32, tag="l2")
                for j in range(JC):
                    ps = ps_mm.tile([128, NCH], F32, tag="mm")
                    for k in range(KC):
                        wi = (p * JC + j) * KC + k
                        nc.tensor.matmul(
                            ps[:], w1_r[:, wi, :], h[:, k, :],
                            start=(k == 0), stop=(k == KC - 1),
                        )
                    nc.scalar.activation(
                        h2[:, j, :], ps[:], RELU,
                        bias=b1_sb[:, p * JC + j : p * JC + j + 1],
                    )
                    if direct:
                        nc.tensor.matmul(
                            r[:], w2_r[:, p * KC + j : p * KC + j + 1],
                            h2[:, j, :], start=(j == 0), stop=(j == JC - 1),
                        )
                        continue
                    nc.vector.tensor_scalar_mul(
                        h2[:, j, :], h2[:, j, :],
                        w2_sb[:, p * KC + j : p * KC + j + 1],
                    )
                    if j == 1:
                        nc.vector.tensor_add(g[:], h2[:, 0, :], h2[:, 1, :])
                    elif j > 1:
                        nc.vector.tensor_add(g[:], g[:], h2[:, j, :])
                if not direct:
                    nc.tensor.matmul(r[:], ones_r[:], g[:], start=True, stop=True)
                o = out_pool.tile([1, NCH], F32, tag="o")
                nc.scalar.activation(o[:], r[:], SIGMOID, bias=b2_sb[0:1, p : p + 1])
                nc.sync.dma_start(out_dram[p : p + 1, n0 : n0 + NCH], o[:])

            def load_et(c, eng=None):
                # et0 rides gpsimd (issues immediately); later chunks ride
                # sync BEHIND the weight pieces so they can't steal wire
                # bandwidth from w1 during the ramp.  During the ramp (c<2)
                # the f32r casts alternate DVE/ACT so they finish sooner.
                n0 = c * NCH
                et_f = et_pool.tile([128, KC, NCH], F32, tag="et")
                eng = eng or nc.sync
                for k in range(KC):
                    eng.dma_start(et_f[:, k, :], eT_v[:, k, n0 : n0 + NCH])
                et = etr_pool.tile([128, KC, NCH], F32R, tag="etr")
                for k in range(KC):
                    if c < 2 and k % 2 == 1:
                        nc.scalar.activation(et[:, k, :], et_f[:, k, :], COPY)
                    else:
                        nc.vector.tensor_copy(et[:, k, :], et_f[:, k, :])
                return et

            # --- chunk 0: L0 for both partitions first (needs only w0+et0),
            # giving the wire time to deliver w1; loads interleaved in
            # consumption order ---
            et0 = load_et(0, eng=nc.gpsimd)
            for q in range(1, P_PER * JC):  # w0 pieces 1..7
                w_dma(w0_st, w0_dram, q)
            for q in range(P_PER * JC):  # all w1 pieces
                w_dma(w1_st, w1_dram, q)
            for q in range(1, P_PER * JC):
                w_round(w0_st, w0_rf, q, alt=(q % 2 == 1))
            h_0 = l0(0, et0)
            h_1 = l0(1, et0)
            et1 = load_et(1)
            for q in range(P_PER * JC):
                w_round(w1_st, w1_rf, q, alt=(q % 2 == 1))
            l12(0, h_0, 0)
            l12(1, h_1, 0)

            # --- steady-state loop ---
            ets = {1: et1}
            for c in range(1, NCHUNKS):
                et = ets.pop(c)
                if c + 1 < NCHUNKS:
                    ets[c + 1] = load_et(c + 1)
                n0 = c * NCH
                for p in range(P_PER):
                    h = l0(p, et)
                    last = c == NCHUNKS - 1 and p == P_PER - 1
                    l12(p, h, n0, direct=last)

    nc.compile()
    return nc


_NC_CACHE = None


def _get_nc():
    global _NC_CACHE
    if _NC_CACHE is None:
        _NC_CACHE = _build()
    return _NC_CACHE


def _make_in_maps(e_embedding, W0, b0, W1, b1, W2, b2):
    e = np.asarray(e_embedding, dtype=np.float32)
    W0 = np.asarray(W0, dtype=np.float32)
    b0 = np.asarray(b0, dtype=np.float32)
    W1 = np.asarray(W1, dtype=np.float32)
    b1 = np.asarray(b1, dtype=np.float32)
    W2 = np.asarray(W2, dtype=np.float32)
    b2 = np.asarray(b2, dtype=np.float32)

    eT = np.ascontiguousarray(e.T)  # [E, N]
    in_maps = []
    for cid in range(N_CORES):
        sl = slice(P_PER * cid, P_PER * (cid + 1))
        # SBUF layout [ki, ((p j k) ji)] — PE consumption order
        w0t = np.ascontiguousarray(
            W0[sl]
            .reshape(P_PER, KC, 128, JC, 128)
            .transpose(2, 0, 3, 1, 4)
            .reshape(128, NW * 128)
        )
        w1t = np.ascontiguousarray(
            W1[sl]
            .reshape(P_PER, KC, 128, JC, 128)
            .transpose(2, 0, 3, 1, 4)
            .reshape(128, NW * 128)
        )
        # [ki, (p j)] bias columns
        b0t = np.ascontiguousarray(
            b0[sl].reshape(P_PER, JC, 128).transpose(2, 0, 1).reshape(128, -1)
        )
        b1t = np.ascontiguousarray(
            b1[sl].reshape(P_PER, JC, 128).transpose(2, 0, 1).reshape(128, -1)
        )
        w2t = np.ascontiguousarray(
            W2[sl, :, 0].reshape(P_PER, KC, 128).transpose(2, 0, 1).reshape(128, -1)
        )
        b2t = np.ascontiguousarray(b2[sl].reshape(1, P_PER))
        in_maps.append(
            {"eT": eT, "w0": w0t, "b0": b0t, "w1": w1t, "b1": b1t,
             "w2": w2t, "b2": b2t}
        )
    return in_maps


def kernel_with_results(trace=False, **inputs):
    nc = _get_nc()
    in_maps = _make_in_maps(**inputs)
    try:
        res = run_bass_kernel_spmd(
            nc, in_maps, core_ids=list(range(N_CORES)), trace=trace
        )
    except Exception:
        # the first PJRT compile in a fresh container can fail transiently;
        # one retry reuses the primed NEFF cache
        res = run_bass_kernel_spmd(
            nc, in_maps, core_ids=list(range(N_CORES)), trace=trace
        )
    full = np.concatenate([r["out"] for r in res.results], axis=0)  # [16, N]
    out = np.ascontiguousarray(full.T).astype(np.float32)  # [N, 16]
    return out, res


def kernel(**inputs):
    out, _ = kernel_with_results(trace=False, **inputs)
    return out
